# revision 1
# baseline (speedup 1.0000x reference)
"""DeepSpeed self-attention layer on 8 Trainium2 NeuronCores.

Sharding: tensor-parallel over heads (2 heads/core), DeepSpeed-mp style.
Per core: full x -> layernorm -> transpose -> fused QKV (its 2 heads) ->
rotary -> causal attention (streaming, no-max-softmax with ones-column
row-sum) -> normalized ctx^T -> AllGather -> output projection on this
core's 512-token slice (partition-id dynamic offset) -> host concat.

Host-side folds: norm_w/norm_b into QKV weights/bias, 1/sqrt(HD) into the
q-side rotary tables, input-mask bias into an extra k^T row.
"""

import numpy as np

import concourse.bass as bass
import concourse.mybir as mybir
import concourse.tile as tile
from concourse import bacc

# Problem shape (hardcoded per contest spec)
B, S, H, NH, HD = 2, 2048, 1024, 16, 64
NCORES = 8
HPC = NH // NCORES          # heads per core = 2
T = B * S                   # 4096 flat tokens
NTILES = T // 128           # 32 token tiles
KC = H // 128               # 8 contraction chunks
TPB = S // 128              # 16 token tiles per batch
GQ = 4                      # q groups of 512 per batch
TPC = T // NCORES           # 512 tokens per core (output slice)
EPS = 1e-5
F32 = mybir.dt.float32
F32R = mybir.dt.float32r


def _r(ap):
    return ap.bitcast(F32R)


def _bc(ap, count, axis=1):
    """Insert a step-0 broadcast dim of size `count` at free-dim position
    `axis` (1 = right after the partition dim)."""
    new = list(ap.ap)
    new.insert(axis, [0, count])
    return bass.AP(tensor=ap.tensor, offset=ap.offset, ap=new)


def build_nc(with_cc=True):
    nc = bacc.Bacc("TRN2", num_devices=NCORES, debug=False)

    x = nc.dram_tensor("x", [T, H], F32, kind="ExternalInput")
    wqkv = nc.dram_tensor("wqkv", [H, 3 * 128], F32R, kind="ExternalInput")
    bqkv = nc.dram_tensor("bqkv", [1, 3 * 128], F32R, kind="ExternalInput")
    cos_q = nc.dram_tensor("cos_q", [128, TPB, HD], F32, kind="ExternalInput")
    sinx_q = nc.dram_tensor("sinx_q", [128, TPB, HD], F32, kind="ExternalInput")
    cos_k = nc.dram_tensor("cos_k", [128, TPB, HD], F32, kind="ExternalInput")
    sinx_k = nc.dram_tensor("sinx_k", [128, TPB, HD], F32, kind="ExternalInput")
    kbias = nc.dram_tensor("kbias", [B, S], F32R, kind="ExternalInput")
    ow = nc.dram_tensor("ow", [H, H], F32R, kind="ExternalInput")
    ob = nc.dram_tensor("ob", [1, H], F32R, kind="ExternalInput")
    identm = nc.dram_tensor("identm", [128, 128], F32R, kind="ExternalInput")
    out_slice = nc.dram_tensor("out_slice", [TPC, H], F32, kind="ExternalOutput")

    with tile.TileContext(nc) as tc:
        with (
            tc.tile_pool(name="singles", bufs=1) as singles,
            tc.tile_pool(name="qkvstore", bufs=1) as qkvstore,
            tc.tile_pool(name="dram", bufs=1, space="DRAM") as dram,
        ):
            # ---- constants ----
            ident = singles.tile([128, 128], F32R)
            nc.sync.dma_start(out=ident, in_=identm[:, :])
            ones1 = singles.tile([1, 128], F32R)
            nc.vector.memset(ones1.bitcast(F32), 1.0)
            eps_t = singles.tile([128, 1], F32)
            nc.vector.memset(eps_t, EPS)
            wqkv_sb = singles.tile([128, KC, 384], F32R)
            nc.sync.dma_start(out=wqkv_sb, in_=wqkv.rearrange("(c p) f -> p c f", p=128))
            bqkv_sb = singles.tile([1, 384], F32R)
            nc.sync.dma_start(out=bqkv_sb, in_=bqkv[:, :])
            tabs = {}
            for name, dr in (("cq", cos_q), ("sq", sinx_q), ("ck", cos_k), ("sk", sinx_k)):
                tabs[name] = singles.tile([128, TPB, HD], F32, name=f"tab_{name}", tag=f"tab_{name}")
                nc.sync.dma_start(out=tabs[name], in_=dr[:, :, :])

            # ---- persistent per-batch q/k/v storage ----
            qT, kT, v_sb = {}, {}, {}
            for b in range(B):
                qT[b] = qkvstore.tile([65, HPC, S], F32R, name=f"qT{b}")
                kT[b] = qkvstore.tile([65, HPC, S], F32R, name=f"kT{b}")
                v_sb[b] = qkvstore.tile([128, TPB, HPC, 65], F32R, name=f"v{b}")
                nc.gpsimd.memset(qT[b][64:65, :, :].bitcast(F32), 1.0)
                nc.gpsimd.memset(v_sb[b][:, :, :, 64:65].bitcast(F32), 1.0)
                kb_flat = bass.AP(tensor=kbias, offset=b * S, ap=[[0, 1], [0, HPC], [1, S]])
                nc.sync.dma_start(out=kT[b][64:65, :, :], in_=kb_flat)

            ctx_local = dram.tile([HPC * HD, T], F32R)
            ctx_all = dram.tile([H, T], F32R)

            def phase1(b, xp, xnp, xntp, statp, rotp, tp_ps, qkv_ps, qkt_ps):
                for st in range(TPB):
                    t = b * TPB + st
                    x_t = xp.tile([128, H], F32, name=f"x_{b}_{st}", tag="x")
                    nc.sync.dma_start(out=x_t, in_=x[t * 128:(t + 1) * 128, :])
                    stats = statp.tile([128, 2, 6], F32, name=f"bns_{b}_{st}", tag="bnstats")
                    nc.vector.bn_stats(out=stats[:, 0, :], in_=x_t[:, 0:512])
                    nc.vector.bn_stats(out=stats[:, 1, :], in_=x_t[:, 512:1024])
                    mv = statp.tile([128, 2], F32, name=f"mv_{b}_{st}", tag="mv")
                    nc.vector.bn_aggr(out=mv, in_=stats)
                    sq = statp.tile([128, 1], F32, name=f"sqr_{b}_{st}", tag="sq")
                    nc.scalar.activation(
                        sq, mv[:, 1:2], mybir.ActivationFunctionType.Sqrt, bias=eps_t[:, 0:1]
                    )
                    rstd = statp.tile([128, 1], F32, name=f"rstd_{b}_{st}", tag="rstd")
                    nc.vector.reciprocal(rstd, sq)
                    nmr = statp.tile([128, 1], F32, name=f"nmr_{b}_{st}", tag="nmr")
                    nc.vector.tensor_scalar(
                        nmr, mv[:, 0:1], rstd[:, 0:1], -1.0,
                        op0=mybir.AluOpType.mult, op1=mybir.AluOpType.mult,
                    )
                    self_phase1_tile(b, st, x_t, rstd[:, 0:1], nmr[:, 0:1],
                                     xnp, xntp, rotp, tp_ps, qkv_ps, qkt_ps)

            def self_phase1_tile(b, st, x_t, rstd, nmr, xnp, xntp, rotp, tp_ps, qkv_ps, qkt_ps):
                if True:
                    xn_t = xnp.tile([128, H], F32R, name=f"xn_{b}_{st}", tag="xn")
                    nc.scalar.activation(
                        xn_t, x_t, mybir.ActivationFunctionType.Identity,
                        bias=nmr[:, 0:1], scale=rstd[:, 0:1],
                    )
                    xnT = xntp.tile([128, KC, 128], F32R, name=f"xnT_{b}_{st}", tag="xnT")
                    for half in range(2):
                        tp = tp_ps.tile([128, 512], F32R, name=f"tp_{b}_{st}_{half}", tag="tp")
                        for i in range(4):
                            c = half * 4 + i
                            nc.tensor.transpose(
                                tp[:, i * 128:(i + 1) * 128],
                                xn_t[:, c * 128:(c + 1) * 128], ident,
                            )
                        nc.scalar.copy(
                            xnT[:, half * 4:(half + 1) * 4, :].rearrange("p c f -> p (c f)"),
                            tp,
                        )
                    qkvp = qkv_ps.tile([128, 384], F32, name=f"qkvp_{b}_{st}", tag="qkvp")
                    for c in range(KC):
                        nc.tensor.matmul(
                            qkvp, xnT[:, c, :], wqkv_sb[:, c, :],
                            start=(c == 0), stop=False,
                        )
                    nc.tensor.matmul(qkvp, ones1, bqkv_sb, start=False, stop=True)

                    for which, (ct, sxt) in (("q", ("cq", "sq")), ("k", ("ck", "sk"))):
                        off = 0 if which == "q" else 128
                        pv = qkvp[:, off:off + 128].rearrange("p (h d) -> p h d", h=HPC)
                        cosb = _bc(tabs[ct][:, st, :], HPC)
                        t1 = rotp.tile([128, HPC, HD], F32R, name=f"t1_{b}_{st}_{which}", tag="t1")
                        nc.vector.tensor_tensor(t1, pv, cosb, op=mybir.AluOpType.mult)
                        qr = rotp.tile([128, HPC, HD], F32R, name=f"qr_{b}_{st}_{which}", tag="qr")
                        pv_swap = bass.AP(
                            tensor=pv.tensor, offset=pv.offset + 32,
                            ap=[pv.ap[0], [64, HPC], [-32, 2], [1, 32]],
                        )
                        qr_v = qr.rearrange("p h (u d) -> p h u d", u=2)
                        sx = tabs[sxt][:, st, :]
                        sx_v = bass.AP(tensor=sx.tensor, offset=sx.offset,
                                       ap=[sx.ap[0], [0, HPC], [32, 2], [1, 32]])
                        nc.vector.tensor_tensor(qr_v, pv_swap, sx_v, op=mybir.AluOpType.mult)
                        nc.vector.tensor_tensor(qr, qr, t1, op=mybir.AluOpType.add)
                        dst = qT[b] if which == "q" else kT[b]
                        for h in range(HPC):
                            tph = qkt_ps.tile([64, 128], F32R, name=f"tph_{b}_{st}_{which}_{h}", tag="tph")
                            nc.tensor.transpose(tph, qr[:, h, :], ident)
                            eng = nc.scalar.copy if h == 0 else nc.vector.tensor_copy
                            eng(dst[0:64, h, st * 128:(st + 1) * 128], tph)
                    nc.scalar.copy(
                        v_sb[b][:, st, :, 0:64],
                        qkvp[:, 256:384].rearrange("p (h d) -> p h d", h=HPC),
                    )

            def attention(b, pp, rp, cstp, sc_ps, ctx_ps, rb_ps):
                for h in range(HPC):
                    for gq in range(GQ):
                        nkt = 4 * (gq + 1)
                        ctxp = ctx_ps.tile([65, 512], F32, name=f"ctxp_{b}_{h}_{gq}", tag="ctxp")
                        for kt in range(nkt):
                            diag = kt >= 4 * gq
                            qoff = (kt - 4 * gq) * 128 if diag else 0
                            sc = sc_ps.tile([128, 512], F32, name=f"sc_{b}_{h}_{gq}_{kt}", tag="sc")
                            nc.tensor.matmul(
                                sc[:, qoff:512],
                                kT[b][:, h, kt * 128:(kt + 1) * 128],
                                qT[b][:, h, gq * 512 + qoff:(gq + 1) * 512],
                                start=True, stop=True,
                            )
                            pb = pp.tile([128, 512], F32R, name=f"pb_{b}_{h}_{gq}_{kt}", tag="pb")
                            nc.scalar.activation(
                                pb[:, qoff:512], sc[:, qoff:512],
                                mybir.ActivationFunctionType.Exp,
                            )
                            if diag:
                                nc.gpsimd.affine_select(
                                    out=pb[:, qoff:qoff + 128],
                                    in_=pb[:, qoff:qoff + 128],
                                    compare_op=mybir.AluOpType.is_ge,
                                    fill=0.0, base=0,
                                    pattern=[[1, 128]], channel_multiplier=-1,
                                )
                            nc.tensor.matmul(
                                ctxp[:, qoff:512],
                                v_sb[b][:, kt, h, :],
                                pb[:, qoff:512],
                                start=(kt == 0), stop=(kt == nkt - 1),
                            )
                        rin = rp.tile([1, 512], F32R, name=f"rin_{b}_{h}_{gq}", tag="rin")
                        with nc.allow_low_precision(reason="fp32r rounding within tolerance"):
                            nc.vector.reciprocal(rin, ctxp[64:65, :])
                        rbc = rb_ps.tile([64, 512], F32, name=f"rbc_{b}_{h}_{gq}", tag="rbc")
                        nc.tensor.matmul(rbc, ones1[:, 0:64], rin, start=True, stop=True)
                        cst = cstp.tile([64, 512], F32R, name=f"cst_{b}_{h}_{gq}", tag="cst")
                        nc.vector.tensor_copy(cst, ctxp[0:64, :])
                        nc.vector.tensor_mul(cst, cst, rbc)
                        nc.sync.dma_start(
                            out=ctx_local[h * 64:(h + 1) * 64,
                                          b * S + gq * 512: b * S + (gq + 1) * 512],
                            in_=cst,
                        )

            # ============ Phase 1(b0) | Attention(b0) + Phase 1(b1) ============
            with (
                tc.tile_pool(name="xp", bufs=3) as xp,
                tc.tile_pool(name="xnp", bufs=2) as xnp,
                tc.tile_pool(name="xntp", bufs=2) as xntp,
                tc.tile_pool(name="statp", bufs=4) as statp,
                tc.tile_pool(name="rotp", bufs=3) as rotp,
                tc.tile_pool(name="tp_ps", bufs=3, space="PSUM") as tp_ps,
                tc.tile_pool(name="qkv_ps", bufs=2, space="PSUM") as qkv_ps,
                tc.tile_pool(name="qkt_ps", bufs=3, space="PSUM") as qkt_ps,
            ):
                phase1(0, xp, xnp, xntp, statp, rotp, tp_ps, qkv_ps, qkt_ps)
                phase1(1, xp, xnp, xntp, statp, rotp, tp_ps, qkv_ps, qkt_ps)

            # load output-projection weights; DMA overlaps attention
            ow_sb = singles.tile([128, KC, H], F32R)
            nc.sync.dma_start(out=ow_sb, in_=ow.rearrange("(c p) f -> p c f", p=128))
            ob_sb = singles.tile([1, H], F32R)
            nc.sync.dma_start(out=ob_sb, in_=ob[:, :])

            # ============ Attention ============
            with (
                tc.tile_pool(name="pp1", bufs=4) as pp1,
                tc.tile_pool(name="rp1", bufs=2) as rp1,
                tc.tile_pool(name="cstp1", bufs=3) as cstp1,
                tc.tile_pool(name="sc_ps1", bufs=4, space="PSUM") as sc_ps1,
                tc.tile_pool(name="rb_ps1", bufs=2, space="PSUM") as rb_ps1,
                tc.tile_pool(name="ctx_ps1", bufs=2, space="PSUM") as ctx_ps1,
            ):
                attention(0, pp1, rp1, cstp1, sc_ps1, ctx_ps1, rb_ps1)
                attention(1, pp1, rp1, cstp1, sc_ps1, ctx_ps1, rb_ps1)

            # ================= Phase 2.5: AllGather =================
            if with_cc:
                nc.gpsimd.collective_compute(
                    "AllGather", mybir.AluOpType.bypass,
                    replica_groups=[list(range(NCORES))],
                    ins=[ctx_local.opt()], outs=[ctx_all.opt()],
                )

            # ================= Phase 3: output projection =================
            with (
                tc.tile_pool(name="cap", bufs=4) as cap,
                tc.tile_pool(name="ostg", bufs=4) as ostg,
                tc.tile_pool(name="op_ps", bufs=4, space="PSUM") as op_ps,
            ):
                pid = nc.partition_id()
                base = pid * TPC
                ctx_r = ctx_all.rearrange("(c p) t -> p c t", p=128)
                ctxAs = []
                for tt in range(TPC // 128):
                    ctxA = cap.tile([128, KC, 128], F32R, name=f"ctxA_{tt}", tag="ctxA")
                    nc.gpsimd.dma_start(
                        out=ctxA, in_=ctx_r[:, :, bass.ds(base + tt * 128, 128)]
                    )
                    ctxAs.append(ctxA)
                for tt in range(TPC // 128):
                    ctxA = ctxAs[tt]
                    for nh in range(2):
                        op = op_ps.tile([128, 512], F32, name=f"op_{tt}_{nh}", tag="op")
                        for c in range(KC):
                            nc.tensor.matmul(
                                op, ctxA[:, c, :], ow_sb[:, c, nh * 512:(nh + 1) * 512],
                                start=(c == 0), stop=False,
                            )
                        nc.tensor.matmul(
                            op, ones1, ob_sb[:, nh * 512:(nh + 1) * 512],
                            start=False, stop=True,
                        )
                        ost = ostg.tile([128, 512], F32, name=f"ost_{tt}_{nh}", tag="ost")
                        (nc.scalar.copy if nh == 0 else nc.vector.tensor_copy)(ost, op)
                        nc.sync.dma_start(
                            out=out_slice[tt * 128:(tt + 1) * 128,
                                          nh * 512:(nh + 1) * 512],
                            in_=ost,
                        )
    nc.compile()
    return nc


def make_inputs(x, input_mask, norm_w, norm_b, attn_qkvw, attn_qkvb, attn_ow, attn_ob):
    """Host preprocessing -> list of per-core input dicts."""
    x = np.asarray(x, np.float32).reshape(T, H)
    input_mask = np.asarray(input_mask)
    norm_w = np.asarray(norm_w, np.float32)
    norm_b = np.asarray(norm_b, np.float32)
    attn_qkvw = np.asarray(attn_qkvw, np.float32)
    attn_qkvb = np.asarray(attn_qkvb, np.float32)
    attn_ow = np.asarray(attn_ow, np.float32)
    attn_ob = np.asarray(attn_ob, np.float32)

    wp = norm_w[:, None] * attn_qkvw                     # fold LN scale
    bp = attn_qkvb + norm_b @ attn_qkvw                  # fold LN shift

    pos = np.arange(S, dtype=np.float32)
    inv_freq = 1.0 / (10000.0 ** (np.arange(0, HD, 2, dtype=np.float32) / HD))
    freqs = pos[:, None] * inv_freq[None, :]             # [S, 32]
    cos_full = np.concatenate([np.cos(freqs)] * 2, -1)   # [S, 64]
    sin_full = np.concatenate([np.sin(freqs)] * 2, -1)
    sinx = sin_full.copy()
    sinx[:, :32] *= -1.0

    def tabify(a):  # [S, 64] -> [128, TPB, 64]
        return np.ascontiguousarray(
            a.reshape(TPB, 128, HD).swapaxes(0, 1).astype(np.float32)
        )

    scale = 1.0 / np.sqrt(HD).astype(np.float32)
    cos_q_t = tabify(cos_full * scale)
    sinx_q_t = tabify(sinx * scale)
    cos_k_t = tabify(cos_full)
    sinx_k_t = tabify(sinx)

    kbias_t = ((1.0 - input_mask.astype(np.float32)) * -10000.0).astype(np.float32)

    in_maps = []
    for c in range(NCORES):
        hs = slice(c * HPC * HD, (c + 1) * HPC * HD)     # this core's 128 cols
        wqkv_c = np.ascontiguousarray(
            np.concatenate([wp[:, hs], wp[:, H:][:, hs], wp[:, 2 * H:][:, hs]], axis=1)
        )
        bqkv_c = np.ascontiguousarray(
            np.concatenate([bp[hs], bp[H:][hs], bp[2 * H:][hs]])[None, :]
        )
        in_maps.append({
            "x": x,
            "identm": np.eye(128, dtype=np.float32),
            "wqkv": wqkv_c,
            "bqkv": bqkv_c,
            "cos_q": cos_q_t, "sinx_q": sinx_q_t,
            "cos_k": cos_k_t, "sinx_k": sinx_k_t,
            "kbias": kbias_t,
            "ow": attn_ow,
            "ob": np.ascontiguousarray(attn_ob[None, :]),
        })
    return in_maps


_CACHE = {}


def _get_runner():
    """Build nc once and return a callable(in_maps) -> list of out dicts,
    reusing one jitted shard_map across calls."""
    if "runner" in _CACHE:
        return _CACHE["runner"]
    import jax
    import jax.numpy as jnp
    from jax.sharding import Mesh, PartitionSpec
    from jax.experimental.shard_map import shard_map
    from concourse import bass2jax
    from concourse import mybir as _mybir

    nc = build_nc()
    bass2jax.install_neuronx_cc_hook()

    partition_name = nc.partition_id_tensor.name if nc.partition_id_tensor else None
    in_names, out_names, out_avals = [], [], []
    for alloc in nc.m.functions[0].allocations:
        if not isinstance(_mybir.MemoryLocationSet, type) or not isinstance(alloc, _mybir.MemoryLocationSet):
            continue
        name = alloc.memorylocations[0].name
        if alloc.kind == "ExternalInput":
            if name != partition_name:
                in_names.append(name)
        elif alloc.kind == "ExternalOutput":
            out_names.append(name)
            out_avals.append(
                jax.core.ShapedArray(tuple(alloc.tensor_shape), _mybir.dt.np(alloc.dtype))
            )
    n_params = len(in_names)
    all_names = in_names + out_names
    if partition_name is not None:
        all_names.append(partition_name)

    def _body(*args):
        operands = list(args)
        if partition_name is not None:
            operands.append(bass2jax.partition_id_tensor())
        outs = bass2jax._bass_exec_p.bind(
            *operands,
            out_avals=tuple(out_avals),
            in_names=tuple(all_names),
            out_names=tuple(out_names),
            lowering_input_output_aliases=(),
            sim_require_finite=True,
            sim_require_nnan=True,
            nc=nc,
        )
        return tuple(outs)

    devices = jax.devices()[:NCORES]
    mesh = Mesh(np.asarray(devices), ("core",))
    n_outs = len(out_names)
    in_specs = (PartitionSpec("core"),) * (n_params + n_outs)
    out_specs = (PartitionSpec("core"),) * n_outs
    sharded = jax.jit(
        shard_map(_body, mesh=mesh, in_specs=in_specs, out_specs=out_specs,
                  check_rep=False),
        keep_unused=True,
    )

    def _body2(*args):
        ins = args[:n_params]
        o1 = _body(*args)
        o2 = _body(*ins, *o1)
        return o2

    sharded2 = jax.jit(
        shard_map(_body2, mesh=mesh, in_specs=in_specs, out_specs=out_specs,
                  check_rep=False),
        keep_unused=True,
    )

    from jax.sharding import NamedSharding
    shard = NamedSharding(mesh, PartitionSpec("core"))

    def to_device(in_maps):
        concat_in = [
            np.concatenate([np.asarray(in_maps[c][nm]) for c in range(NCORES)], axis=0)
            for nm in in_names
        ]
        concat_zeros = [
            np.zeros((NCORES * a.shape[0], *a.shape[1:]), a.dtype) for a in out_avals
        ]
        return [jax.device_put(a, shard) for a in concat_in + concat_zeros]

    def run_device(dev_args):
        out_arrs = sharded(*dev_args)
        jax.block_until_ready(out_arrs)
        return out_arrs

    def run_device2(dev_args):
        out_arrs = sharded2(*dev_args)
        jax.block_until_ready(out_arrs)
        return out_arrs

    def runner(in_maps):
        out_arrs = run_device(to_device(in_maps))
        return [
            {nm: np.asarray(out_arrs[i]).reshape(NCORES, *out_avals[i].shape)[c]
             for i, nm in enumerate(out_names)}
            for c in range(NCORES)
        ]

    runner.to_device = to_device
    runner.run_device = run_device
    runner.run_device2 = run_device2
    _CACHE["runner"] = runner
    return runner


def kernel(**inputs) -> np.ndarray:
    in_maps = make_inputs(**inputs)
    runner = _get_runner()
    results = runner(in_maps)
    full = np.concatenate([results[c]["out_slice"] for c in range(NCORES)], axis=0)
    return full.reshape(B, S, H).astype(np.float32)

